# revision 38
# baseline (speedup 1.0000x reference)
"""Trainium2 Bass kernel for the combined loss (KL + CE + InfoNCE + focal + adv CE).

v2 strategy (8 NeuronCores, data-parallel over the batch):
  - InfoNCE exploits Gram symmetry.  The 8192x8192 cosine-similarity matrix is
    a 64x64 grid of 128x128 blocks.  Every block-row r computes only the blocks
    at circulant distance d = 0..32 (columns (r+d) mod 64).  Each computed
    exp-block feeds TWO row sums: its own rows via the ScalarEngine's fused
    exp + row-accumulate, and the mirrored rows (distance 64-d) via a
    ones-matmul column-sum accumulated in PSUM by the PE.  This halves both
    the exp work (the baseline bottleneck) and the Gram matmuls.
  - Features are normalized on the host and shipped as bf16 [256, 8704]
    (columns rolled per core by 512*c and extended by 512 so the circulant
    sweep is contiguous; one SPMD program serves all cores).
  - CE/KL/focal/adv: per-sample stats ([128,1] accumulators) are computed
    on-device; the tiny per-row nonlinear epilogue (log/focal buckets) runs
    on the host on the gathered 4096-row stats.
  - Each core ships ~70KB of partial sums; the host reduces and applies the
    loss weights.
"""

import numpy as np
import ml_dtypes

import concourse.bacc as bacc
import concourse.tile as tile
from concourse import mybir
from concourse.bass_utils import run_bass_kernel_spmd

F32 = mybir.dt.float32
BF16 = mybir.dt.bfloat16
AF = mybir.ActivationFunctionType
ALU = mybir.AluOpType
AX = mybir.AxisListType

NCORES = 8
B, C, D = 4096, 1000, 256
RB = B // NCORES          # 512 rows of the [B, C] tensors per core
NT = RB // 128            # 4 row-tiles per core
N2 = 2 * B                # 8192 infoNCE rows
NDIST = 33                # circulant distances d = 0..32 per block-row
SPAN = NDIST * 128        # 4224 columns per block-row sweep
L_ROWS = [0, 1, 2, 3, 32, 33, 34, 35]   # local block-row indices (all cores)
EXT = 35 * 128 + SPAN     # 8704 extended local columns
CHW = 1536                # gram/exp chunk width (3 PSUM banks)
NCHK = (EXT + CHW - 1) // CHW           # 6 chunks
RGW = 512                 # colsum accumulation region width (1 PSUM bank)
NREG = EXT // RGW         # 17 regions

KL_TEMP = 4.0
KL_INTERP = 0.5
NCE_TEMP = 0.07
NEG_BIG = -1.0e9


def _pair_table():
    """(l, c, a, b) for every (block-row, chunk) intersection, chunk-major."""
    pairs = []
    for c in range(NCHK):
        c0, c1 = CHW * c, min(CHW * (c + 1), EXT)
        for l in L_ROWS:
            s, e = 128 * l, 128 * l + SPAN
            a, b = max(c0, s), min(c1, e)
            if b > a:
                pairs.append((l, c, a, b))
    return pairs


PAIRS = _pair_table()
NSLOT = len(PAIRS)        # 28


def _region_pieces():
    """region -> list of (pair_idx, p0, p1) colsum pieces, in emission order."""
    reg = {}
    for idx, (l, c, a, b) in enumerate(PAIRS):
        a2 = max(a, 128 * l + 128)        # exclude d=0 (diagonal block)
        b2 = min(b, 128 * l + 4096)       # exclude d=32 (rowsum-only block)
        if b2 <= a2:
            continue
        r0, r1 = a2 // RGW, (b2 - 1) // RGW
        for r in range(r0, r1 + 1):
            p0, p1 = max(a2, RGW * r), min(b2, RGW * (r + 1))
            reg.setdefault(r, []).append((idx, p0, p1))
    return reg


REGION_PIECES = _region_pieces()


def _build_module():
    nc = bacc.Bacc("TRN2", target_bir_lowering=False, debug=False)

    # packed per-tile CE/KL input: o(f32) | m(bf16) | a(f32) | tg | ta bytes
    OMA_W = 4 * C + 2 * C + 4 * C + 32
    oma_d = nc.dram_tensor("oma", [RB, OMA_W], mybir.dt.uint8,
                           kind="ExternalInput")
    ft_d = nc.dram_tensor("ft", [256, EXT], BF16, kind="ExternalInput")
    out_d = nc.dram_tensor("out", [128, 61], F32, kind="ExternalOutput")
    cs_d = nc.dram_tensor("cs", [1, EXT], F32, kind="ExternalOutput")

    iota_np = np.tile(np.arange(C, dtype=np.float32), (128, 1))
    # packed constants: ident f32 | identb bf16 | negidb | onesb | zerob
    cpack = np.concatenate([
        np.eye(128, dtype=np.float32).view(np.uint8).reshape(128, -1),
        np.eye(128).astype(ml_dtypes.bfloat16).view(np.uint8).reshape(128, -1),
        (NEG_BIG * np.eye(128)).astype(ml_dtypes.bfloat16).view(np.uint8).reshape(128, -1),
        np.ones((128, 128)).astype(ml_dtypes.bfloat16).view(np.uint8).reshape(128, -1),
        np.zeros((128, RGW)).astype(ml_dtypes.bfloat16).view(np.uint8).reshape(128, -1),
    ], axis=1)
    iota_d = nc.inline_tensor(iota_np, "iota_c")
    cpack_d = nc.inline_tensor(cpack, "cpack_c")

    with tile.TileContext(nc) as tc:
        with (
            tc.tile_pool(name="persist", bufs=1) as persist,
            tc.tile_pool(name="io", bufs=1) as iop,
            tc.tile_pool(name="scr", bufs=2) as scrp,
            tc.tile_pool(name="et", bufs=12) as etp,
            tc.tile_pool(name="vec", bufs=1) as vecp,
            tc.tile_pool(name="gp", bufs=2, space="PSUM") as gpp,
            tc.tile_pool(name="cs", bufs=2, space="PSUM") as csp,
        ):
            dma = nc.sync.dma_start

            cpack_t = persist.tile([128, cpack.shape[1]], mybir.dt.uint8,
                                   tag="cpack")
            ident_t = cpack_t[:, 0:512].bitcast(F32)
            identb_t = cpack_t[:, 512:768].bitcast(BF16)
            negidb_t = cpack_t[:, 768:1024].bitcast(BF16)
            onesb_t = cpack_t[:, 1024:1280].bitcast(BF16)
            zerob_t = cpack_t[:, 1280:1280 + 2 * RGW].bitcast(BF16)

            h0b = persist.tile([128, EXT], BF16, tag="h0b")
            h1b = persist.tile([128, EXT], BF16, tag="h1b")
            iota_t = persist.tile([128, C], F32, tag="iota")
            o_ts, m_ts, a_ts, oma_ts = [], [], [], []
            for t in range(NT):
                oma_t = iop.tile([128, OMA_W], mybir.dt.uint8, tag=f"oma{t}")
                oma_ts.append(oma_t)
                o_ts.append(oma_t[:, 0:4 * C].bitcast(F32))
                m_ts.append(oma_t[:, 4 * C:6 * C].bitcast(BF16))
                a_ts.append(oma_t[:, 6 * C:10 * C].bitcast(F32))
            tg_t = oma_ts[0][:, 10 * C:10 * C + 16].bitcast(F32)
            ta_t = oma_ts[0][:, 10 * C + 16:10 * C + 32].bitcast(F32)

            # progressive feature pieces so the first gram starts early
            P0, P1, P2 = 512, 1536, 4608
            dma(out=h0b[:, 0:P0], in_=ft_d[0:128, 0:P0])
            dma(out=h1b[:, 0:P0], in_=ft_d[128:256, 0:P0])
            dma(out=cpack_t[:], in_=cpack_d[:])
            dma(out=h0b[:, P0:P1], in_=ft_d[0:128, P0:P1])
            dma(out=h1b[:, P0:P1], in_=ft_d[128:256, P0:P1])
            dma(out=h0b[:, P1:P2], in_=ft_d[0:128, P1:P2])
            dma(out=h1b[:, P1:P2], in_=ft_d[128:256, P1:P2])
            dma(out=iota_t[:], in_=iota_d[:])
            for t in range(NT):
                rsl = slice(t * 128, (t + 1) * 128)
                dma(out=oma_ts[t][:], in_=oma_d[rsl, :])
                if t == 0:
                    dma(out=h0b[:, P2:EXT], in_=ft_d[0:128, P2:EXT])
                    dma(out=h1b[:, P2:EXT], in_=ft_d[128:256, P2:EXT])

            out_sb = vecp.tile([128, 61], F32, tag="out_sb")
            rs_sl = out_sb[:, 0:NSLOT]
            rs_x = out_sb[:, 28:29]          # second slot of the split pair 0
            pos_sb = out_sb[:, 29:33]
            st_sb = out_sb[:, 33:61]
            cs_sb = vecp.tile([1, EXT], F32, tag="cs_sb")
            pscr = vecp.tile([128, 128], F32, tag="pscr")
            dummy = vecp.tile([128, CHW], BF16, tag="dummy")

            et_tiles = {}          # pair idx -> exp tile handle (per chunk)

            em_ts = {}

            def emit_cekl_act(t, half):
                # two ~2.9us ACT bursts per tile so the PE never idles past
                # the HAM MID window during a CE/KL slot
                if half == 0:
                    nc.scalar.activation(dummy[:, :C], o_ts[t][:], AF.Exp,
                                         scale=1.0,
                                         accum_out=st_sb[:, 0 + t:1 + t])
                    nc.scalar.activation(dummy[:, :C], o_ts[t][:], AF.Exp,
                                         scale=float(1.0 / KL_TEMP),
                                         accum_out=st_sb[:, 4 + t:5 + t])
                else:
                    em_t = scrp.tile([128, C], BF16, tag="em")
                    em_ts[t] = em_t
                    nc.scalar.activation(em_t[:], m_ts[t][:], AF.Exp,
                                         scale=float(1.0 / KL_TEMP),
                                         accum_out=st_sb[:, 8 + t:9 + t])
                    nc.scalar.activation(dummy[:, :C], a_ts[t][:], AF.Exp,
                                         scale=1.0,
                                         accum_out=st_sb[:, 12 + t:13 + t])

            def emit_cekl_dve(t):
                d_t = scrp.tile([128, C], BF16, tag="d")
                nc.vector.tensor_sub(d_t[:], m_ts[t][:], o_ts[t][:])
                nc.vector.scalar_tensor_tensor(
                    out=dummy[:, :C], in0=d_t[:], scalar=1.0, in1=em_ts[t][:],
                    op0=ALU.mult, op1=ALU.mult,
                    accum_out=st_sb[:, 16 + t:17 + t])
                nc.vector.scalar_tensor_tensor(
                    out=dummy[:, :C], in0=iota_t[:], scalar=tg_t[:, t:t + 1],
                    in1=o_ts[t][:], op0=ALU.is_equal, op1=ALU.mult,
                    accum_out=st_sb[:, 20 + t:21 + t])
                nc.vector.scalar_tensor_tensor(
                    out=dummy[:, :C], in0=iota_t[:], scalar=ta_t[:, t:t + 1],
                    in1=a_ts[t][:], op0=ALU.is_equal, op1=ALU.mult,
                    accum_out=st_sb[:, 24 + t:25 + t])

            def emit_gram_exp(idx):
                l, c, a, b = PAIRS[idx]
                w = b - a
                s_l = 128 * l
                gp = gpp.tile([128, CHW], F32, tag="gp")
                if idx == 0:
                    # fast path: close region [0,512) after 3 matmuls so the
                    # very first exp fires as soon as the 512-col DMA lands
                    nc.tensor.matmul(gp[:, 0:512], h0b[:, 0:128],
                                     h0b[:, 0:512], start=True, stop=False,
                                     skip_group_check=True)
                    nc.tensor.matmul(gp[:, 0:128], negidb_t[:], identb_t[:],
                                     start=False, stop=False,
                                     skip_group_check=True)
                    nc.tensor.matmul(gp[:, 0:512], h1b[:, 0:128],
                                     h1b[:, 0:512], start=False, stop=True,
                                     skip_group_check=True)
                    e_t = etp.tile([128, CHW], BF16, tag="et")
                    et_tiles[idx] = e_t
                    nc.scalar.activation(e_t[:, 0:512], gp[:, 0:512], AF.Exp,
                                         scale=float(1.0 / NCE_TEMP),
                                         accum_out=rs_x[:])
                    for half, hb in ((0, h0b), (1, h1b)):
                        for sub in range(512, w, 512):
                            n = min(512, w - sub)
                            nc.tensor.matmul(gp[:, sub:sub + n],
                                             hb[:, 0:128],
                                             hb[:, sub:sub + n],
                                             start=(half == 0),
                                             stop=(half == 1),
                                             skip_group_check=True)
                    nc.scalar.activation(e_t[:, 512:w], gp[:, 512:w], AF.Exp,
                                         scale=float(1.0 / NCE_TEMP),
                                         accum_out=rs_sl[:, 0:1])
                    return
                # gram: all h0 sub-matmuls (start), then h1 (stop), so the
                # stationary operand only swaps once per half
                for half, hb in ((0, h0b), (1, h1b)):
                    for sub in range(0, w, 512):
                        n = min(512, w - sub)
                        nc.tensor.matmul(gp[:, sub:sub + n],
                                         hb[:, s_l:s_l + 128],
                                         hb[:, a + sub:a + sub + n],
                                         start=(half == 0), stop=(half == 1),
                                         skip_group_check=True)
                    if half == 0 and a == s_l:
                        # mask the self-similarity diagonal
                        nc.tensor.matmul(gp[:, 0:128], negidb_t[:], identb_t[:],
                                         start=False, stop=False,
                                         skip_group_check=True)
                # positive-pair logits: diagonal of the d=32 block (l<4 only)
                p0 = s_l + 4096
                if l < 4 and a <= p0 < b:
                    off = p0 - a
                    nc.vector.scalar_tensor_tensor(
                        out=pscr[:], in0=gp[:, off:off + 128], scalar=1.0,
                        in1=ident_t[:], op0=ALU.mult, op1=ALU.mult,
                        accum_out=pos_sb[:, l:l + 1])
                e_t = etp.tile([128, CHW], BF16, tag="et")
                et_tiles[idx] = e_t
                nc.scalar.activation(e_t[:, :w], gp[:, :w], AF.Exp,
                                     scale=float(1.0 / NCE_TEMP),
                                     accum_out=rs_sl[:, idx:idx + 1])

            def emit_colsums(c):
                # mirrored row sums: one 512-wide PSUM accumulator at a time
                for r in range(3 * c, min(3 * c + 3, NREG)):
                    if r not in REGION_PIECES:
                        continue
                    pieces = REGION_PIECES[r]
                    ct = csp.tile([128, RGW], F32, tag="cs")
                    full0 = pieces[0][1] == RGW * r and pieces[0][2] == RGW * (r + 1)
                    if not full0:
                        nc.tensor.matmul(ct[:], onesb_t[:], zerob_t[:],
                                         start=True, stop=False,
                                         skip_group_check=True)
                    for k, (idx, p0_, p1_) in enumerate(pieces):
                        _, _, a, _ = PAIRS[idx]
                        nc.tensor.matmul(
                            ct[:, p0_ - RGW * r:p1_ - RGW * r],
                            onesb_t[:], et_tiles[idx][:, p0_ - a:p1_ - a],
                            start=(k == 0 and full0), stop=(k == len(pieces) - 1),
                            skip_group_check=True)
                    nc.vector.tensor_copy(
                        cs_sb[0:1, RGW * r:RGW * (r + 1)], ct[0:1, :])
                # stream colsums out every second chunk
                if c % 2 == 1 or c == NCHK - 1:
                    lo = RGW * 3 * (c - 1 if c % 2 == 1 else c)
                    hi = min(RGW * 3 * (c + 1), EXT)
                    dma(out=cs_d[0:1, lo:hi], in_=cs_sb[0:1, lo:hi])

            # interleave NCE chunks with CE/KL half-tiles so ACT never starves
            pair_of_chunk = [[i for i, p in enumerate(PAIRS) if p[1] == c]
                             for c in range(NCHK)]
            act_slots = [(t, h) for t in range(NT) for h in (0, 1)]
            dve_slots = list(range(NT))
            for c in range(NCHK):
                # DVE stat work for a tile whose ACT halves are both done
                if dve_slots and len(act_slots) <= 2 * NT - 2 * (
                        dve_slots[0] + 1):
                    emit_cekl_dve(dve_slots.pop(0))
                for k, idx in enumerate(pair_of_chunk[c]):
                    emit_gram_exp(idx)
                    if c >= 1 and k == len(pair_of_chunk[c]) // 2 and act_slots:
                        emit_cekl_act(*act_slots.pop(0))
                emit_colsums(c)
                if c >= 1 and act_slots:
                    emit_cekl_act(*act_slots.pop(0))
            while dve_slots:
                emit_cekl_dve(dve_slots.pop(0))

            dma(out=out_d[:], in_=out_sb[:])

    nc.compile()
    return nc


_NC = None


def _get_nc():
    global _NC
    if _NC is None:
        _NC = _build_module()
    return _NC


def _prep_inputs(output, target, master_net_pred, feat_pooled,
                 feat_pooled_masked, output_adv, target_adv):
    o = np.ascontiguousarray(np.asarray(output, dtype=np.float32))
    m = np.asarray(master_net_pred, dtype=np.float32)
    a = np.ascontiguousarray(np.asarray(output_adv, dtype=np.float32))
    tg = np.asarray(target).astype(np.int64)
    ta = np.asarray(target_adv).astype(np.int64)
    f0 = np.asarray(feat_pooled, dtype=np.float32)
    f1 = np.asarray(feat_pooled_masked, dtype=np.float32)
    feats = np.concatenate([f0, f1], axis=0)  # [2B, D]
    feats = feats / np.linalg.norm(feats, axis=1, keepdims=True)
    m_bf = m.astype(ml_dtypes.bfloat16)

    in_maps = []
    for cc in range(NCORES):
        sl = slice(cc * RB, (cc + 1) * RB)
        rolled = np.roll(feats, -RB * cc, axis=0)
        ext = np.concatenate([rolled, rolled[:EXT - N2]], axis=0)  # [8704, D]
        ftc = np.ascontiguousarray(ext.T.astype(ml_dtypes.bfloat16))
        tgta = np.zeros((RB, 32), dtype=np.uint8)
        tgc = np.ascontiguousarray(
            tg[sl].reshape(NT, 128).T.astype(np.float32))
        tac = np.ascontiguousarray(
            ta[sl].reshape(NT, 128).T.astype(np.float32))
        tgta[0:128, 0:16] = tgc.view(np.uint8).reshape(128, 16)
        tgta[0:128, 16:32] = tac.view(np.uint8).reshape(128, 16)
        oma = np.concatenate([
            o[sl].view(np.uint8).reshape(RB, -1),
            np.ascontiguousarray(m_bf[sl]).view(np.uint8).reshape(RB, -1),
            a[sl].view(np.uint8).reshape(RB, -1),
            tgta,
        ], axis=1)
        in_maps.append({"oma": np.ascontiguousarray(oma), "ft": ftc})
    return in_maps


def _combine(results):
    S = np.zeros(N2, dtype=np.float64)
    pos_full = np.zeros(N2, dtype=np.float64)
    arp = np.arange(128)
    for cc, rr in enumerate(results):
        rs = rr["out"][:, 0:NSLOT].astype(np.float64)
        rs[:, 0] += rr["out"][:, 28].astype(np.float64)
        cs = rr["cs"].reshape(-1).astype(np.float64)   # [EXT]
        pos = rr["out"][:, 29:33].astype(np.float64)
        for idx, (l, c, a, b) in enumerate(PAIRS):
            rows = (RB * cc + 128 * l + arp) % N2
            np.add.at(S, rows, rs[:, idx])
        gcols = (np.arange(EXT) + RB * cc) % N2
        np.add.at(S, gcols, cs)
        for l in range(4):
            i = RB * cc + 128 * l + arp
            pos_full[i] = pos[:, l]
            pos_full[i + B] = pos[:, l]
    nce_mean = float(np.mean(np.log(S) - pos_full / NCE_TEMP))

    # CE / KL / focal / adv from per-row stats
    sts = [r["out"][:, 33:61] for r in results]
    S1 = np.concatenate([st[:, 0:4].T.reshape(-1) for st in sts])
    ST = np.concatenate([st[:, 4:8].T.reshape(-1) for st in sts])
    SM = np.concatenate([st[:, 8:12].T.reshape(-1) for st in sts])
    SA = np.concatenate([st[:, 12:16].T.reshape(-1) for st in sts])
    PP = np.concatenate([st[:, 16:20].T.reshape(-1) for st in sts])
    GO = np.concatenate([st[:, 20:24].T.reshape(-1) for st in sts])
    GA = np.concatenate([st[:, 24:28].T.reshape(-1) for st in sts])
    S1, ST, SM, SA, PP, GO, GA = (x.astype(np.float64)
                                  for x in (S1, ST, SM, SA, PP, GO, GA))
    ce = np.log(S1) - GO
    adv = np.log(SA) - GA
    kl = PP / (KL_TEMP * SM) - np.log(SM) + np.log(ST)
    pt = np.exp(-ce)
    gamma = np.where(pt < 0.2, 5.0, np.where(pt < 0.5, 3.0, 1.0))
    foc = ((1.0 - pt) ** gamma) * ce
    loss = (KL_INTERP * KL_TEMP * KL_TEMP) * np.mean(kl) / C \
        + (1.0 - KL_INTERP) * np.mean(ce) + nce_mean \
        + np.mean(foc) + np.mean(adv)
    return np.asarray([loss], dtype=np.float32)


def kernel(**inputs):
    in_maps = _prep_inputs(**inputs)
    out = run_bass_kernel_spmd(_get_nc(), in_maps,
                               core_ids=list(range(NCORES)))
    return _combine(out.results)


if __name__ == "__main__":
    rng = np.random.default_rng(0)
    ins = {
        "output": rng.standard_normal((B, C), dtype=np.float32),
        "target": rng.integers(0, C, size=(B,)),
        "master_net_pred": rng.standard_normal((B, C), dtype=np.float32),
        "feat_pooled": rng.standard_normal((B, D), dtype=np.float32),
        "feat_pooled_masked": rng.standard_normal((B, D), dtype=np.float32),
        "output_adv": rng.standard_normal((B, C), dtype=np.float32),
        "target_adv": rng.integers(0, C, size=(B,)),
    }
    print(kernel(**ins))
